# revision 7
# baseline (speedup 1.0000x reference)
"""Multi-head attention (B=8, N=1024, C=768, H=12) on 8 TRN2 NeuronCores.

Sharding: data-parallel over batch — core i computes batch element i fully.
Weights / bias table are replicated. No collectives.

Per-core kernel (all matmuls bf16, f32 PSUM accumulation):
  1. qkv projection from host-pretransposed x^T (c-major):
       Q^T,K^T: [d-part, token-free], head pairs packed 2x64 per 128 partitions
       V:       [token-part, d-free], stored as lhsT tiles [128j, 65] with the
                key-padding mask folded into the V rows and a mask-valued
                "ones" column appended (column 64).
  2. attention, flipped orientation S^T[j,i] (keys on partitions):
       S^T = (K^T)^T-matmul, then P = exp(S^T) * emtab, where emtab is a
       host-precomputed shifted-Toeplitz table of exp(rel_pos_bias) so the
       bias-add becomes a multiply (exp(S+B) = exp(S)exp(B)) and needs no
       row-max subtraction (|S| <= ~10 is safe in f32/bf16 exp).
       O^T_unnorm[d,i] and denom[i] come from ONE matmul per (jt, i-chunk):
       lhsT = [V|mask] so PSUM row 64 accumulates the masked softmax sum.
  3. normalize lazily: recip(denoms) once, broadcast via DRAM bounce, one
     multiply per head-pair slot.
  4. output projection back to [token-part, feature-free] + bias, DMA out f32.
"""

import numpy as np
import ml_dtypes

DIM = 768
NUM_HEADS = 12
HD = 64
N_TOK = 1024
B = 8
SCALE = HD ** -0.5

_BUILD_CACHE = {}


def _build_nc(N=N_TOK, H=NUM_HEADS, mmdt_name="bfloat16"):
    import concourse.bass as bass
    import concourse.mybir as mybir
    import concourse.tile as tile
    from concourse import bacc

    f32 = mybir.dt.float32
    mmdt = getattr(mybir.dt, mmdt_name)
    Exp = mybir.ActivationFunctionType.Exp
    mult = mybir.AluOpType.mult
    add = mybir.AluOpType.add

    C = H * HD                      # 768
    NT = N // 128                   # token tiles (j and i)
    KO = C // 128                   # contraction slots (== head pairs HP)
    HP = H // 2
    TW = 2 * N - 128                # shifted exp-table width
    ichunks = [(i0, min(512, N - i0)) for i0 in range(0, N, 512)]
    fchunks = [(f0, min(512, C - f0)) for f0 in range(0, C, 512)]

    nc = bacc.Bacc(None)
    xT_d = nc.declare_dram_parameter("xT", [C, N], mmdt, isOutput=False)
    wqk_d = nc.declare_dram_parameter("wqk", [C, 2 * C], mmdt, isOutput=False)
    wv_d = nc.declare_dram_parameter("wv", [C, C], mmdt, isOutput=False)
    wp_d = nc.declare_dram_parameter("wp", [C, C], mmdt, isOutput=False)
    emtab_d = nc.declare_dram_parameter("emtab", [H, 128, TW], mmdt, isOutput=False)
    mask_d = nc.declare_dram_parameter("maskr", [128, NT], mmdt, isOutput=False)
    bp_d = nc.declare_dram_parameter("bproj", [C], f32, isOutput=False)
    out_d = nc.declare_dram_parameter("out", [N, C], f32, isOutput=True)

    with tile.TileContext(nc) as tc:
        with (
            tc.tile_pool(name="singles", bufs=1) as singles,
            tc.tile_pool(name="dram", bufs=1, space="DRAM") as drampool,
        ):
            xT = singles.tile([128, KO, N], mmdt)
            nc.sync.dma_start(xT[:], xT_d.rearrange("(ko p) n -> p ko n", p=128))
            wqk = singles.tile([128, KO, 2 * C], mmdt)
            nc.sync.dma_start(wqk[:], wqk_d.rearrange("(ko p) m -> p ko m", p=128))
            wv = singles.tile([128, KO, C], mmdt)
            nc.sync.dma_start(wv[:], wv_d.rearrange("(ko p) m -> p ko m", p=128))
            wp = singles.tile([128, KO, C], mmdt)
            nc.sync.dma_start(wp[:], wp_d.rearrange("(ko p) m -> p ko m", p=128))
            maskr = singles.tile([128, NT], mmdt)
            nc.sync.dma_start(maskr[:], mask_d[:])
            bp = singles.tile([128, C], f32)
            nc.sync.dma_start(
                bp[:],
                bass.AP(tensor=bp_d, offset=0, ap=[[0, 128], [1, C]]),
            )

            qt = singles.tile([128, HP, N], mmdt)
            kt = singles.tile([128, HP, N], mmdt)
            vsb = singles.tile([128, NT, H, HD + 1], mmdt)
            ou = singles.tile([128, HP, N], mmdt)      # unnormalized O^T (packed)
            den = singles.tile([128, N], f32)          # row h = head h denominators
            rdb = singles.tile([128, N], mmdt)         # bf16 recip denominators
            rb = singles.tile([128, HP, N], mmdt)      # broadcast recips (packed)
            rscratch = drampool.tile([H, N], mmdt)

            # ---------------- Phase 1: qkv projections ----------------
            with tc.tile_pool(name="qkv_psum", bufs=4, space="PSUM") as qp:
                # Q^T / K^T  (features on partitions)
                for mt in range(H):
                    dst = qt if mt < HP else kt
                    slot = mt % HP
                    for i0, il in ichunks:
                        ps = qp.tile([128, 512], f32, tag="ps")
                        for ko in range(KO):
                            nc.tensor.matmul(
                                ps[:, :il],
                                lhsT=wqk[:, ko, 128 * mt : 128 * mt + 128],
                                rhs=xT[:, ko, i0 : i0 + il],
                                start=(ko == 0),
                                stop=(ko == KO - 1),
                            )
                        nc.vector.tensor_copy(
                            dst[:, slot, i0 : i0 + il], ps[:, :il]
                        )
                # V (tokens on partitions), masked, written into [V|mask] tiles
                for jt in range(NT):
                    for f0, fl in fchunks:
                        ps = qp.tile([128, 512], f32, tag="ps")
                        for ko in range(KO):
                            nc.tensor.matmul(
                                ps[:, :fl],
                                lhsT=xT[:, ko, 128 * jt : 128 * jt + 128],
                                rhs=wv[:, ko, f0 : f0 + fl],
                                start=(ko == 0),
                                stop=(ko == KO - 1),
                            )
                        h0, nh = f0 // HD, fl // HD
                        nc.vector.tensor_tensor(
                            vsb[:, jt, h0 : h0 + nh, 0:HD],
                            ps[:, :fl].rearrange("p (h d) -> p h d", d=HD),
                            maskr[:, jt : jt + 1, None].to_broadcast([128, nh, HD]),
                            mult,
                        )
                    # mask column (the "ones" column that accumulates denom)
                    nc.vector.tensor_scalar_mul(
                        vsb[:, jt, :, HD : HD + 1],
                        maskr[:, jt : jt + 1, None].to_broadcast([128, H, 1]),
                        1.0,
                    )

            # ---------------- Phase 2: attention per head ----------------
            with (
                tc.tile_pool(name="emt_pool", bufs=2) as emt_pool,
                tc.tile_pool(name="st_psum", bufs=2, space="PSUM") as st_psum,
                tc.tile_pool(name="pv_psum", bufs=2 * len(ichunks), space="PSUM") as pv_psum,
                tc.tile_pool(name="e_pool", bufs=3) as e_pool,
                tc.tile_pool(name="p_pool", bufs=3) as p_pool,
                tc.tile_pool(name="drow_pool", bufs=3) as drow_pool,
            ):
                for h in range(H):
                    hp, ho = h // 2, 64 * (h % 2)
                    emt = emt_pool.tile([128, TW], mmdt, tag="emt")
                    nc.sync.dma_start(emt[:], emtab_d[h])
                    pvs = [
                        pv_psum.tile([128, 512], f32, tag="pv", name=f"pv_{h}_{ic}")
                        for ic in range(len(ichunks))
                    ]
                    for jt in range(NT):
                        st = st_psum.tile([128, N], f32, tag="st")
                        for i0, il in ichunks:
                            nc.tensor.matmul(
                                st[:, i0 : i0 + il],
                                lhsT=kt[ho : ho + 64, hp, 128 * jt : 128 * jt + 128],
                                rhs=qt[ho : ho + 64, hp, i0 : i0 + il],
                                start=True,
                                stop=True,
                            )
                        e = e_pool.tile([128, N], mmdt, tag="e")
                        nc.scalar.activation(e[:], st[:], Exp)
                        p = p_pool.tile([128, N], mmdt, tag="p")
                        base = 128 * (NT - 1 - jt)
                        nc.vector.tensor_tensor(
                            p[:], e[:], emt[:, base : base + N], mult
                        )
                        for ic, (i0, il) in enumerate(ichunks):
                            nc.tensor.matmul(
                                pvs[ic][: HD + 1, :il],
                                lhsT=vsb[:, jt, h, :],
                                rhs=p[:, i0 : i0 + il],
                                start=(jt == 0),
                                stop=(jt == NT - 1),
                            )
                    for ic, (i0, il) in enumerate(ichunks):
                        nc.vector.tensor_copy(
                            ou[ho : ho + 64, hp, i0 : i0 + il],
                            pvs[ic][:HD, :il],
                        )
                        drow = drow_pool.tile(
                            [128, 512], f32, tag="drow", name=f"drow_{h}_{ic}"
                        )
                        nc.vector.tensor_copy(
                            drow[64:65, :il], pvs[ic][HD : HD + 1, :il]
                        )
                        nc.sync.dma_start(
                            den[h : h + 1, i0 : i0 + il], drow[64:65, :il]
                        )

            # ---------------- normalize ----------------
            rden = singles.tile([128, N], f32)
            nc.vector.reciprocal(rden[:H], den[:H])
            nc.vector.tensor_copy(rdb[:H], rden[:H])
            nc.sync.dma_start(rscratch[:], rdb[:H])
            for h in range(H):
                hp, ho = h // 2, 64 * (h % 2)
                nc.sync.dma_start(
                    rb[ho : ho + 64, hp, :],
                    bass.AP(
                        tensor=rscratch.tensor,
                        offset=rscratch[h, 0].offset,
                        ap=[[0, 64], [1, N]],
                    ),
                )
            for hp in range(HP):
                nc.vector.tensor_tensor(
                    ou[:, hp, :], ou[:, hp, :], rb[:, hp, :], mult
                )

            # ---------------- Phase 3: output projection ----------------
            with (
                tc.tile_pool(name="proj_psum", bufs=3, space="PSUM") as proj_psum,
                tc.tile_pool(name="o_pool", bufs=3) as o_pool,
            ):
                for it in range(NT):
                    ot = o_pool.tile([128, C], f32, tag="ot")
                    for f0, fl in fchunks:
                        ps = proj_psum.tile([128, 512], f32, tag="ps")
                        for ko in range(KO):
                            nc.tensor.matmul(
                                ps[:, :fl],
                                lhsT=ou[:, ko, 128 * it : 128 * it + 128],
                                rhs=wp[:, ko, f0 : f0 + fl],
                                start=(ko == 0),
                                stop=(ko == KO - 1),
                            )
                        nc.vector.tensor_tensor(
                            ot[:, f0 : f0 + fl], ps[:, :fl], bp[:, f0 : f0 + fl], add
                        )
                    nc.sync.dma_start(out_d[128 * it : 128 * it + 128, :], ot[:])

    nc.finalize()
    return nc


def _host_pack(x, w_qkv, w_proj, b_proj, bias_table, key_padding_mask,
               N=N_TOK, H=NUM_HEADS, mmdt_name="bfloat16"):
    """Host-side layout: per-core input dicts (core i <- batch i)."""
    np_mmdt = ml_dtypes.bfloat16 if mmdt_name == "bfloat16" else np.float32
    C = H * HD
    NT = N // 128
    TW = 2 * N - 128

    w_qkv = np.asarray(w_qkv, np.float32)
    wqk = np.ascontiguousarray(w_qkv[: 2 * C].T).astype(np.float32)
    wqk[:, :C] *= SCALE                       # fold softmax scale into W_q
    wqk = wqk.astype(np_mmdt)
    wv = np.ascontiguousarray(w_qkv[2 * C :].T).astype(np_mmdt)
    wp = np.ascontiguousarray(np.asarray(w_proj, np.float32).T).astype(np_mmdt)

    etab = np.exp(np.asarray(bias_table, np.float32))          # [2N-1, H]
    idx = (TW - 1) + np.arange(128)[:, None] - np.arange(TW)[None, :]
    emtab = np.ascontiguousarray(etab[idx, :].transpose(2, 0, 1)).astype(np_mmdt)

    bp = np.asarray(b_proj, np.float32)
    x = np.asarray(x, np.float32)
    mask = np.asarray(key_padding_mask)

    in_maps = []
    for b in range(x.shape[0]):
        xT = np.ascontiguousarray(x[b].T).astype(np_mmdt)
        mr = np.ascontiguousarray(
            mask[b].astype(np.float32).reshape(NT, 128).T
        ).astype(np_mmdt)
        in_maps.append({
            "xT": xT, "wqk": wqk, "wv": wv, "wp": wp,
            "emtab": emtab, "maskr": mr, "bproj": bp,
        })
    return in_maps


def _run(x, w_qkv, w_proj, b_proj, bias_table, key_padding_mask, trace=False):
    from concourse.bass_utils import run_bass_kernel_spmd

    key = ("full", N_TOK, NUM_HEADS)
    if key not in _BUILD_CACHE:
        _BUILD_CACHE[key] = _build_nc()
    nc = _BUILD_CACHE[key]
    in_maps = _host_pack(x, w_qkv, w_proj, b_proj, bias_table, key_padding_mask)
    res = run_bass_kernel_spmd(nc, in_maps, core_ids=list(range(B)), trace=trace)
    out = np.stack([np.asarray(res.results[i]["out"]) for i in range(B)])
    return out.astype(np.float32), res


def kernel(x, w_qkv, w_proj, b_proj, bias_table, key_padding_mask):
    out, _ = _run(x, w_qkv, w_proj, b_proj, bias_table, key_padding_mask)
    return out


# revision 13
# speedup vs baseline: 1.3560x; 1.3560x over previous
"""Multi-head attention (B=8, N=1024, C=768, H=12) on 8 TRN2 NeuronCores.

Sharding: data-parallel over batch — core i computes batch element i fully.
Weights / bias table are replicated. No collectives.

Per-core kernel (all matmuls bf16, f32 PSUM accumulation):
  1. qkv projection from host-pretransposed x^T (c-major):
       Q^T,K^T: [d-part, token-free], head pairs packed 2x64 per 128 partitions
       V:       [token-part, d-free], stored as lhsT tiles [128j, 65] with the
                key-padding mask folded into the V rows and a mask-valued
                "ones" column appended (column 64).
  2. attention, flipped orientation S^T[j,i] (keys on partitions):
       S^T = (K^T)^T-matmul, then P = exp(S^T) * emtab, where emtab is a
       host-precomputed shifted-Toeplitz table of exp(rel_pos_bias) so the
       bias-add becomes a multiply (exp(S+B) = exp(S)exp(B)) and needs no
       row-max subtraction (|S| <= ~10 is safe in f32/bf16 exp).
       O^T_unnorm[d,i] and denom[i] come from ONE matmul per (jt, i-chunk):
       lhsT = [V|mask] so PSUM row 64 accumulates the masked softmax sum.
  3. normalize lazily: recip(denoms) once, broadcast via DRAM bounce, one
     multiply per head-pair slot.
  4. output projection back to [token-part, feature-free] + bias, DMA out f32.
"""

import numpy as np
import ml_dtypes

DIM = 768
NUM_HEADS = 12
HD = 64
N_TOK = 1024
B = 8
SCALE = HD ** -0.5

_BUILD_CACHE = {}


def _build_nc(N=N_TOK, H=NUM_HEADS, mmdt_name="bfloat16"):
    import concourse.bass as bass
    import concourse.mybir as mybir
    import concourse.tile as tile
    from concourse import bacc

    f32 = mybir.dt.float32
    mmdt = getattr(mybir.dt, mmdt_name)
    Exp = mybir.ActivationFunctionType.Exp
    mult = mybir.AluOpType.mult
    add = mybir.AluOpType.add

    C = H * HD                      # 768
    NT = N // 128                   # token tiles (j and i)
    KO = C // 128                   # contraction slots (== head pairs HP)
    HP = H // 2
    TW = 2 * N - 128                # shifted exp-table width
    ichunks = [(i0, min(512, N - i0)) for i0 in range(0, N, 512)]
    fchunks = [(f0, min(512, C - f0)) for f0 in range(0, C, 512)]

    nc = bacc.Bacc(None)
    xT_d = nc.declare_dram_parameter("xT", [C, N], mmdt, isOutput=False)
    wqk_d = nc.declare_dram_parameter("wqk", [C, 2 * C], mmdt, isOutput=False)
    wv_d = nc.declare_dram_parameter("wv", [C, C], mmdt, isOutput=False)
    wp_d = nc.declare_dram_parameter("wp", [C, C], mmdt, isOutput=False)
    emtab_d = nc.declare_dram_parameter("emtab", [H, 128, TW], mmdt, isOutput=False)
    mask_d = nc.declare_dram_parameter("maskr", [128, NT], mmdt, isOutput=False)
    bp_d = nc.declare_dram_parameter("bproj", [C], f32, isOutput=False)
    out_d = nc.declare_dram_parameter("out", [N, C], f32, isOutput=True)

    with tile.TileContext(nc) as tc:
        with (
            tc.tile_pool(name="singles", bufs=1) as singles,
            tc.tile_pool(name="dram", bufs=1, space="DRAM") as drampool,
        ):
            xT = singles.tile([128, KO, N], mmdt)
            nc.sync.dma_start(xT[:], xT_d.rearrange("(ko p) n -> p ko n", p=128))
            wqk = singles.tile([128, KO, 2 * C], mmdt)
            nc.sync.dma_start(wqk[:], wqk_d.rearrange("(ko p) m -> p ko m", p=128))
            wv = singles.tile([128, KO, C], mmdt)
            nc.sync.dma_start(wv[:], wv_d.rearrange("(ko p) m -> p ko m", p=128))
            wp = singles.tile([128, KO, C], mmdt)
            nc.sync.dma_start(wp[:], wp_d.rearrange("(ko p) m -> p ko m", p=128))
            maskr = singles.tile([128, NT], mmdt)
            nc.sync.dma_start(maskr[:], mask_d[:])
            bp = singles.tile([128, C], f32)
            nc.sync.dma_start(
                bp[:],
                bass.AP(tensor=bp_d, offset=0, ap=[[0, 128], [1, C]]),
            )

            qt = singles.tile([128, HP, N], mmdt)
            kt = singles.tile([128, HP, N], mmdt)
            vsb = singles.tile([128, NT, H, HD + 1], mmdt)
            ou = singles.tile([128, HP, N], mmdt)      # unnormalized O^T (packed)
            den = singles.tile([128, N], f32)          # row h = head h denominators
            rdb = singles.tile([128, N], mmdt)         # bf16 recip denominators
            rb = singles.tile([128, HP, N], mmdt)      # broadcast recips (packed)
            rscratch = drampool.tile([H, N], mmdt)

            # -------- fused pipeline: V proj, then per head-pair QK proj +
            # -------- attention (lag-2 software-pipelined PV) + normalize ----
            rden = singles.tile([128, N], f32)
            with (
                tc.tile_pool(name="qkv_psum", bufs=2, space="PSUM") as qp,
                tc.tile_pool(name="emt_pool", bufs=2) as emt_pool,
                tc.tile_pool(name="st_psum", bufs=2, space="PSUM") as st_psum,
                tc.tile_pool(name="pv_psum", bufs=len(ichunks), space="PSUM") as pv_psum,
                tc.tile_pool(name="e_pool", bufs=3) as e_pool,
                tc.tile_pool(name="p_pool", bufs=4) as p_pool,
                tc.tile_pool(name="drow_pool", bufs=3) as drow_pool,
            ):
                # V (tokens on partitions), masked, written into [V|mask] tiles
                for jt in range(NT):
                    for f0, fl in fchunks:
                        ps = qp.tile([128, 512], f32, tag="ps")
                        for ko in range(KO):
                            nc.tensor.matmul(
                                ps[:, :fl],
                                lhsT=xT[:, ko, 128 * jt : 128 * jt + 128],
                                rhs=wv[:, ko, f0 : f0 + fl],
                                start=(ko == 0),
                                stop=(ko == KO - 1),
                            )
                        h0, nh = f0 // HD, fl // HD
                        nc.vector.tensor_tensor(
                            vsb[:, jt, h0 : h0 + nh, 0:HD],
                            ps[:, :fl].rearrange("p (h d) -> p h d", d=HD),
                            maskr[:, jt : jt + 1, None].to_broadcast([128, nh, HD]),
                            mult,
                        )
                    # mask column (the "ones" column that accumulates denom)
                    nc.vector.tensor_scalar_mul(
                        vsb[:, jt, :, HD : HD + 1],
                        maskr[:, jt : jt + 1, None].to_broadcast([128, H, 1]),
                        1.0,
                    )

                GP1 = (HP + 1) // 2          # pairs in normalize group 0

                def _normalize_group(g0, r0, ng):
                    nc.vector.reciprocal(rden[r0 : r0 + ng], den[r0 : r0 + ng])
                    nc.vector.tensor_copy(rdb[r0 : r0 + ng], rden[r0 : r0 + ng])
                    nc.sync.dma_start(rscratch[g0 : g0 + ng], rdb[r0 : r0 + ng])
                    for h in range(g0, g0 + ng):
                        ho = 64 * (h % 2)
                        nc.sync.dma_start(
                            rb[ho : ho + 64, h // 2, :],
                            bass.AP(
                                tensor=rscratch.tensor,
                                offset=rscratch[h, 0].offset,
                                ap=[[0, 64], [1, N]],
                            ),
                        )
                    for sl in range(g0 // 2, (g0 + ng) // 2):
                        nc.vector.tensor_tensor(
                            ou[:, sl, :], ou[:, sl, :], rb[:, sl, :], mult
                        )

                for pair in range(HP):
                    # Q^T / K^T projections for this head pair
                    for mt in (pair, HP + pair):
                        dst = qt if mt < HP else kt
                        for i0, il in ichunks:
                            ps = qp.tile([128, 512], f32, tag="ps")
                            for ko in range(KO):
                                nc.tensor.matmul(
                                    ps[:, :il],
                                    lhsT=wqk[:, ko, 128 * mt : 128 * mt + 128],
                                    rhs=xT[:, ko, i0 : i0 + il],
                                    start=(ko == 0),
                                    stop=(ko == KO - 1),
                                )
                            nc.vector.tensor_copy(
                                dst[:, pair, i0 : i0 + il], ps[:, :il]
                            )
                    # attention for the pair's two heads
                    for h in (2 * pair, 2 * pair + 1):
                        hp, ho = pair, 64 * (h % 2)
                        emt = emt_pool.tile([128, TW], mmdt, tag="emt")
                        nc.sync.dma_start(emt[:], emtab_d[h])
                        pvs = [
                            pv_psum.tile([128, 512], f32, tag="pv", name=f"pv_{h}_{ic}")
                            for ic in range(len(ichunks))
                        ]
                        ptiles = {}
                        for jt in range(NT + 2):
                            if jt < NT:
                                st = st_psum.tile([128, N], f32, tag="st")
                                for i0, il in ichunks:
                                    nc.tensor.matmul(
                                        st[:, i0 : i0 + il],
                                        lhsT=kt[ho : ho + 64, hp, 128 * jt : 128 * jt + 128],
                                        rhs=qt[ho : ho + 64, hp, i0 : i0 + il],
                                        start=True,
                                        stop=True,
                                    )
                                e = e_pool.tile([128, N], mmdt, tag="e")
                                nc.scalar.activation(e[:], st[:], Exp)
                                p = p_pool.tile([128, N], mmdt, tag="p")
                                base = 128 * (NT - 1 - jt)
                                nc.vector.tensor_tensor(
                                    p[:], e[:], emt[:, base : base + N], mult
                                )
                                ptiles[jt] = p
                            if jt >= 2:
                                jd = jt - 2
                                pd = ptiles.pop(jd)
                                for ic, (i0, il) in enumerate(ichunks):
                                    nc.tensor.matmul(
                                        pvs[ic][: HD + 1, :il],
                                        lhsT=vsb[:, jd, h, :],
                                        rhs=pd[:, i0 : i0 + il],
                                        start=(jd == 0),
                                        stop=(jd == NT - 1),
                                    )
                        for ic, (i0, il) in enumerate(ichunks):
                            nc.vector.tensor_copy(
                                ou[ho : ho + 64, hp, i0 : i0 + il],
                                pvs[ic][:HD, :il],
                            )
                            drow = drow_pool.tile(
                                [128, 512], f32, tag="drow", name=f"drow_{h}_{ic}"
                            )
                            nc.vector.tensor_copy(
                                drow[64:65, :il], pvs[ic][HD : HD + 1, :il]
                            )
                            dr = h if h < 2 * GP1 else 32 + h - 2 * GP1
                            nc.sync.dma_start(
                                den[dr : dr + 1, i0 : i0 + il], drow[64:65, :il]
                            )
                    # normalize in two groups of head-pairs (32-aligned
                    # partition starts for the DVE ops); overlaps later compute
                    if pair in (GP1 - 1, HP - 1) and not (
                        pair == HP - 1 and GP1 == HP
                    ):
                        g0 = 0 if pair == GP1 - 1 else 2 * GP1
                        r0 = 0 if pair == GP1 - 1 else 32
                        ng = (2 * GP1) if pair == GP1 - 1 else (H - 2 * GP1)
                        _normalize_group(g0, r0, ng)
                    elif pair == HP - 1:
                        _normalize_group(0, 0, H)

            # ---------------- Phase 3: output projection ----------------
            with (
                tc.tile_pool(name="proj_psum", bufs=3, space="PSUM") as proj_psum,
                tc.tile_pool(name="o_pool", bufs=3) as o_pool,
            ):
                for it in range(NT):
                    ot = o_pool.tile([128, C], f32, tag="ot")
                    for f0, fl in fchunks:
                        ps = proj_psum.tile([128, 512], f32, tag="ps")
                        for ko in range(KO):
                            nc.tensor.matmul(
                                ps[:, :fl],
                                lhsT=ou[:, ko, 128 * it : 128 * it + 128],
                                rhs=wp[:, ko, f0 : f0 + fl],
                                start=(ko == 0),
                                stop=(ko == KO - 1),
                            )
                        nc.vector.tensor_tensor(
                            ot[:, f0 : f0 + fl], ps[:, :fl], bp[:, f0 : f0 + fl], add
                        )
                    nc.sync.dma_start(out_d[128 * it : 128 * it + 128, :], ot[:])

    nc.finalize()
    return nc


def _host_pack(x, w_qkv, w_proj, b_proj, bias_table, key_padding_mask,
               N=N_TOK, H=NUM_HEADS, mmdt_name="bfloat16"):
    """Host-side layout: per-core input dicts (core i <- batch i)."""
    np_mmdt = ml_dtypes.bfloat16 if mmdt_name == "bfloat16" else np.float32
    C = H * HD
    NT = N // 128
    TW = 2 * N - 128

    w_qkv = np.asarray(w_qkv, np.float32)
    wqk = np.ascontiguousarray(w_qkv[: 2 * C].T).astype(np.float32)
    wqk[:, :C] *= SCALE                       # fold softmax scale into W_q
    wqk = wqk.astype(np_mmdt)
    wv = np.ascontiguousarray(w_qkv[2 * C :].T).astype(np_mmdt)
    wp = np.ascontiguousarray(np.asarray(w_proj, np.float32).T).astype(np_mmdt)

    etab = np.exp(np.asarray(bias_table, np.float32))          # [2N-1, H]
    idx = (TW - 1) + np.arange(128)[:, None] - np.arange(TW)[None, :]
    emtab = np.ascontiguousarray(etab[idx, :].transpose(2, 0, 1)).astype(np_mmdt)

    bp = np.asarray(b_proj, np.float32)
    x = np.asarray(x, np.float32)
    mask = np.asarray(key_padding_mask)

    in_maps = []
    for b in range(x.shape[0]):
        xT = np.ascontiguousarray(x[b].T).astype(np_mmdt)
        mr = np.ascontiguousarray(
            mask[b].astype(np.float32).reshape(NT, 128).T
        ).astype(np_mmdt)
        in_maps.append({
            "xT": xT, "wqk": wqk, "wv": wv, "wp": wp,
            "emtab": emtab, "maskr": mr, "bproj": bp,
        })
    return in_maps


def _run(x, w_qkv, w_proj, b_proj, bias_table, key_padding_mask, trace=False):
    from concourse.bass_utils import run_bass_kernel_spmd

    key = ("full", N_TOK, NUM_HEADS)
    if key not in _BUILD_CACHE:
        _BUILD_CACHE[key] = _build_nc()
    nc = _BUILD_CACHE[key]
    in_maps = _host_pack(x, w_qkv, w_proj, b_proj, bias_table, key_padding_mask)
    res = run_bass_kernel_spmd(nc, in_maps, core_ids=list(range(B)), trace=trace)
    out = np.stack([np.asarray(res.results[i]["out"]) for i in range(B)])
    return out.astype(np.float32), res


def kernel(x, w_qkv, w_proj, b_proj, bias_table, key_padding_mask):
    out, _ = _run(x, w_qkv, w_proj, b_proj, bias_table, key_padding_mask)
    return out


# revision 14
# speedup vs baseline: 1.4793x; 1.0910x over previous
"""Multi-head attention (B=8, N=1024, C=768, H=12) on 8 TRN2 NeuronCores.

Sharding: data-parallel over batch — core i computes batch element i fully.
Weights / bias table are replicated. No collectives.

Per-core kernel (all matmuls bf16, f32 PSUM accumulation):
  1. qkv projection from host-pretransposed x^T (c-major):
       Q^T,K^T: [d-part, token-free], head pairs packed 2x64 per 128 partitions
       V:       [token-part, d-free], stored as lhsT tiles [128j, 65] with the
                key-padding mask folded into the V rows and a mask-valued
                "ones" column appended (column 64).
  2. attention, flipped orientation S^T[j,i] (keys on partitions):
       S^T = (K^T)^T-matmul, then P = exp(S^T) * emtab, where emtab is a
       host-precomputed shifted-Toeplitz table of exp(rel_pos_bias) so the
       bias-add becomes a multiply (exp(S+B) = exp(S)exp(B)) and needs no
       row-max subtraction (|S| <= ~10 is safe in f32/bf16 exp).
       O^T_unnorm[d,i] and denom[i] come from ONE matmul per (jt, i-chunk):
       lhsT = [V|mask] so PSUM row 64 accumulates the masked softmax sum.
  3. normalize lazily: recip(denoms) once, broadcast via DRAM bounce, one
     multiply per head-pair slot.
  4. output projection back to [token-part, feature-free] + bias, DMA out f32.
"""

import numpy as np
import ml_dtypes

DIM = 768
NUM_HEADS = 12
HD = 64
N_TOK = 1024
B = 8
SCALE = HD ** -0.5

_BUILD_CACHE = {}


def _build_nc(N=N_TOK, H=NUM_HEADS, mmdt_name="bfloat16"):
    import concourse.bass as bass
    import concourse.mybir as mybir
    import concourse.tile as tile
    from concourse import bacc

    f32 = mybir.dt.float32
    mmdt = getattr(mybir.dt, mmdt_name)
    Exp = mybir.ActivationFunctionType.Exp
    Ln = mybir.ActivationFunctionType.Ln
    mult = mybir.AluOpType.mult
    add = mybir.AluOpType.add

    C = H * HD                      # 768
    NT = N // 128                   # token tiles (j and i)
    KO = C // 128                   # contraction slots (== head pairs HP)
    HP = H // 2
    TW = 2 * N - 128                # shifted exp-table width
    ichunks = [(i0, min(512, N - i0)) for i0 in range(0, N, 512)]
    fchunks = [(f0, min(512, C - f0)) for f0 in range(0, C, 512)]

    nc = bacc.Bacc(None)
    xT_d = nc.declare_dram_parameter("xT", [C, N], mmdt, isOutput=False)
    wqk_d = nc.declare_dram_parameter("wqk", [C, 2 * C], mmdt, isOutput=False)
    wv_d = nc.declare_dram_parameter("wv", [C, C], mmdt, isOutput=False)
    wp_d = nc.declare_dram_parameter("wp", [C, C], mmdt, isOutput=False)
    emtab_d = nc.declare_dram_parameter("emtab", [H, 128, TW], mmdt, isOutput=False)
    mask_d = nc.declare_dram_parameter("maskr", [128, NT], mmdt, isOutput=False)
    bp_d = nc.declare_dram_parameter("bproj", [C], f32, isOutput=False)
    out_d = nc.declare_dram_parameter("out", [N, C], f32, isOutput=True)

    with tile.TileContext(nc) as tc:
        with (
            tc.tile_pool(name="singles", bufs=1) as singles,
            tc.tile_pool(name="dram", bufs=1, space="DRAM") as drampool,
        ):
            KH = KO // 2 if KO >= 2 else KO
            xT = singles.tile([128, KO, N], mmdt)
            xT_r = xT_d.rearrange("(ko p) n -> p ko n", p=128)
            nc.sync.dma_start(xT[:, :KH], xT_r[:, :KH])
            wv = singles.tile([128, KO, C], mmdt)
            nc.sync.dma_start(wv[:], wv_d.rearrange("(ko p) m -> p ko m", p=128))
            maskr = singles.tile([128, NT], mmdt)
            nc.sync.dma_start(maskr[:], mask_d[:])
            if KH < KO:
                nc.sync.dma_start(xT[:, KH:], xT_r[:, KH:])
            wqk = singles.tile([128, KO, 2 * C], mmdt)
            nc.sync.dma_start(wqk[:], wqk_d.rearrange("(ko p) m -> p ko m", p=128))
            wp = singles.tile([128, KO, C], mmdt)
            nc.sync.dma_start(wp[:], wp_d.rearrange("(ko p) m -> p ko m", p=128))
            bp = singles.tile([128, C], f32)
            nc.sync.dma_start(
                bp[:],
                bass.AP(tensor=bp_d, offset=0, ap=[[0, 128], [1, C]]),
            )

            qt = singles.tile([128, HP, N], mmdt)
            kt = singles.tile([128, HP, N], mmdt)
            vsb = singles.tile([128, NT, H, HD + 1], mmdt)
            ou = singles.tile([128, HP, N], mmdt)      # unnormalized O^T (packed)
            den = singles.tile([128, N], f32)          # row h = head h denominators
            rdb = singles.tile([128, N], mmdt)         # bf16 recip denominators
            rb = singles.tile([128, HP, N], mmdt)      # broadcast recips (packed)
            rscratch = drampool.tile([H, N], mmdt)

            # -------- fused pipeline: V proj, then per head-pair QK proj +
            # -------- attention (lag-2 software-pipelined PV) + normalize ----
            rden = singles.tile([128, N], f32)
            with (
                tc.tile_pool(name="qkv_psum", bufs=2, space="PSUM") as qp,
                tc.tile_pool(name="emt_pool", bufs=2) as emt_pool,
                tc.tile_pool(name="st_psum", bufs=2, space="PSUM") as st_psum,
                tc.tile_pool(name="pv_psum", bufs=len(ichunks), space="PSUM") as pv_psum,
                tc.tile_pool(name="e_pool", bufs=3) as e_pool,
                tc.tile_pool(name="p_pool", bufs=4) as p_pool,
                tc.tile_pool(name="drow_pool", bufs=3) as drow_pool,
            ):
                # V (tokens on partitions), masked, written into [V|mask] tiles
                for jt in range(NT):
                    for f0, fl in fchunks:
                        ps = qp.tile([128, 512], f32, tag="ps")
                        for ko in range(KO):
                            nc.tensor.matmul(
                                ps[:, :fl],
                                lhsT=xT[:, ko, 128 * jt : 128 * jt + 128],
                                rhs=wv[:, ko, f0 : f0 + fl],
                                start=(ko == 0),
                                stop=(ko == KO - 1),
                            )
                        h0, nh = f0 // HD, fl // HD
                        nc.vector.tensor_tensor(
                            vsb[:, jt, h0 : h0 + nh, 0:HD],
                            ps[:, :fl].rearrange("p (h d) -> p h d", d=HD),
                            maskr[:, jt : jt + 1, None].to_broadcast([128, nh, HD]),
                            mult,
                        )
                    # mask column (the "ones" column that accumulates denom)
                    nc.vector.tensor_scalar_mul(
                        vsb[:, jt, :, HD : HD + 1],
                        maskr[:, jt : jt + 1, None].to_broadcast([128, H, 1]),
                        1.0,
                    )

                GP1 = (HP + 1) // 2          # pairs in normalize group 0

                def _normalize_group(g0, r0, ng):
                    nc.scalar.activation(
                        rden[r0 : r0 + ng], den[r0 : r0 + ng], Ln
                    )
                    nc.scalar.activation(
                        rdb[r0 : r0 + ng], rden[r0 : r0 + ng], Exp, scale=-1.0
                    )
                    nc.sync.dma_start(rscratch[g0 : g0 + ng], rdb[r0 : r0 + ng])
                    for h in range(g0, g0 + ng):
                        ho = 64 * (h % 2)
                        nc.sync.dma_start(
                            rb[ho : ho + 64, h // 2, :],
                            bass.AP(
                                tensor=rscratch.tensor,
                                offset=rscratch[h, 0].offset,
                                ap=[[0, 64], [1, N]],
                            ),
                        )
                    for sl in range(g0 // 2, (g0 + ng) // 2):
                        nc.vector.tensor_tensor(
                            ou[:, sl, :], ou[:, sl, :], rb[:, sl, :], mult
                        )

                for pair in range(HP):
                    # Q^T / K^T projections for this head pair
                    for mt in (pair, HP + pair):
                        dst = qt if mt < HP else kt
                        for i0, il in ichunks:
                            ps = qp.tile([128, 512], f32, tag="ps")
                            for ko in range(KO):
                                nc.tensor.matmul(
                                    ps[:, :il],
                                    lhsT=wqk[:, ko, 128 * mt : 128 * mt + 128],
                                    rhs=xT[:, ko, i0 : i0 + il],
                                    start=(ko == 0),
                                    stop=(ko == KO - 1),
                                )
                            nc.vector.tensor_copy(
                                dst[:, pair, i0 : i0 + il], ps[:, :il]
                            )
                    # attention for the pair's two heads
                    for h in (2 * pair, 2 * pair + 1):
                        hp, ho = pair, 64 * (h % 2)
                        emt = emt_pool.tile([128, TW], mmdt, tag="emt")
                        nc.sync.dma_start(emt[:], emtab_d[h])
                        pvs = [
                            pv_psum.tile([128, 512], f32, tag="pv", name=f"pv_{h}_{ic}")
                            for ic in range(len(ichunks))
                        ]
                        ptiles = {}
                        for jt in range(NT + 2):
                            if jt < NT:
                                st = st_psum.tile([128, N], f32, tag="st")
                                for i0, il in ichunks:
                                    nc.tensor.matmul(
                                        st[:, i0 : i0 + il],
                                        lhsT=kt[ho : ho + 64, hp, 128 * jt : 128 * jt + 128],
                                        rhs=qt[ho : ho + 64, hp, i0 : i0 + il],
                                        start=True,
                                        stop=True,
                                    )
                                e = e_pool.tile([128, N], mmdt, tag="e")
                                nc.scalar.activation(e[:], st[:], Exp)
                                p = p_pool.tile([128, N], mmdt, tag="p")
                                base = 128 * (NT - 1 - jt)
                                nc.vector.tensor_tensor(
                                    p[:], e[:], emt[:, base : base + N], mult
                                )
                                ptiles[jt] = p
                            if jt >= 2:
                                jd = jt - 2
                                pd = ptiles.pop(jd)
                                for ic, (i0, il) in enumerate(ichunks):
                                    nc.tensor.matmul(
                                        pvs[ic][: HD + 1, :il],
                                        lhsT=vsb[:, jd, h, :],
                                        rhs=pd[:, i0 : i0 + il],
                                        start=(jd == 0),
                                        stop=(jd == NT - 1),
                                    )
                        for ic, (i0, il) in enumerate(ichunks):
                            nc.vector.tensor_copy(
                                ou[ho : ho + 64, hp, i0 : i0 + il],
                                pvs[ic][:HD, :il],
                            )
                            drow = drow_pool.tile(
                                [128, 512], f32, tag="drow", name=f"drow_{h}_{ic}"
                            )
                            nc.scalar.copy(
                                drow[64:65, :il], pvs[ic][HD : HD + 1, :il]
                            )
                            dr = h if h < 2 * GP1 else 32 + h - 2 * GP1
                            nc.sync.dma_start(
                                den[dr : dr + 1, i0 : i0 + il], drow[64:65, :il]
                            )
                    # normalize in two groups of head-pairs (32-aligned
                    # partition starts for the DVE ops); overlaps later compute
                    if pair in (GP1 - 1, HP - 1) and not (
                        pair == HP - 1 and GP1 == HP
                    ):
                        g0 = 0 if pair == GP1 - 1 else 2 * GP1
                        r0 = 0 if pair == GP1 - 1 else 32
                        ng = (2 * GP1) if pair == GP1 - 1 else (H - 2 * GP1)
                        _normalize_group(g0, r0, ng)
                    elif pair == HP - 1:
                        _normalize_group(0, 0, H)

            # ---------------- Phase 3: output projection ----------------
            with (
                tc.tile_pool(name="proj_psum", bufs=3, space="PSUM") as proj_psum,
                tc.tile_pool(name="o_pool", bufs=3) as o_pool,
            ):
                for it in range(NT):
                    ot = o_pool.tile([128, C], f32, tag="ot")
                    for f0, fl in fchunks:
                        ps = proj_psum.tile([128, 512], f32, tag="ps")
                        for ko in range(KO):
                            nc.tensor.matmul(
                                ps[:, :fl],
                                lhsT=ou[:, ko, 128 * it : 128 * it + 128],
                                rhs=wp[:, ko, f0 : f0 + fl],
                                start=(ko == 0),
                                stop=(ko == KO - 1),
                            )
                        nc.vector.tensor_tensor(
                            ot[:, f0 : f0 + fl], ps[:, :fl], bp[:, f0 : f0 + fl], add
                        )
                    nc.sync.dma_start(out_d[128 * it : 128 * it + 128, :], ot[:])

    nc.finalize()
    return nc


def _host_pack(x, w_qkv, w_proj, b_proj, bias_table, key_padding_mask,
               N=N_TOK, H=NUM_HEADS, mmdt_name="bfloat16"):
    """Host-side layout: per-core input dicts (core i <- batch i)."""
    np_mmdt = ml_dtypes.bfloat16 if mmdt_name == "bfloat16" else np.float32
    C = H * HD
    NT = N // 128
    TW = 2 * N - 128

    w_qkv = np.asarray(w_qkv, np.float32)
    wqk = np.ascontiguousarray(w_qkv[: 2 * C].T).astype(np.float32)
    wqk[:, :C] *= SCALE                       # fold softmax scale into W_q
    wqk = wqk.astype(np_mmdt)
    wv = np.ascontiguousarray(w_qkv[2 * C :].T).astype(np_mmdt)
    wp = np.ascontiguousarray(np.asarray(w_proj, np.float32).T).astype(np_mmdt)

    etab = np.exp(np.asarray(bias_table, np.float32))          # [2N-1, H]
    idx = (TW - 1) + np.arange(128)[:, None] - np.arange(TW)[None, :]
    emtab = np.ascontiguousarray(etab[idx, :].transpose(2, 0, 1)).astype(np_mmdt)

    bp = np.asarray(b_proj, np.float32)
    x = np.asarray(x, np.float32)
    mask = np.asarray(key_padding_mask)

    in_maps = []
    for b in range(x.shape[0]):
        xT = np.ascontiguousarray(x[b].T).astype(np_mmdt)
        mr = np.ascontiguousarray(
            mask[b].astype(np.float32).reshape(NT, 128).T
        ).astype(np_mmdt)
        in_maps.append({
            "xT": xT, "wqk": wqk, "wv": wv, "wp": wp,
            "emtab": emtab, "maskr": mr, "bproj": bp,
        })
    return in_maps


def _run(x, w_qkv, w_proj, b_proj, bias_table, key_padding_mask, trace=False):
    from concourse.bass_utils import run_bass_kernel_spmd

    key = ("full", N_TOK, NUM_HEADS)
    if key not in _BUILD_CACHE:
        _BUILD_CACHE[key] = _build_nc()
    nc = _BUILD_CACHE[key]
    in_maps = _host_pack(x, w_qkv, w_proj, b_proj, bias_table, key_padding_mask)
    res = run_bass_kernel_spmd(nc, in_maps, core_ids=list(range(B)), trace=trace)
    out = np.stack([np.asarray(res.results[i]["out"]) for i in range(B)])
    return out.astype(np.float32), res


def kernel(x, w_qkv, w_proj, b_proj, bias_table, key_padding_mask):
    out, _ = _run(x, w_qkv, w_proj, b_proj, bias_table, key_padding_mask)
    return out
